# revision 37
# baseline (speedup 1.0000x reference)
"""Trainium2 Bass kernel for nn_AsymetricKernel (linear attention w/ InstanceNorm + 2D rotary).

Sharding: 8 cores = 4 batches x 2 head-groups (4 heads each). Fully independent
per core -- no collectives.

v9 design notes (all PE compute bf16, fp32 PSUM accumulation):
  - InstanceNorm mean subtraction folded into host-centered Wk/Wv; variance is
    sumsq/64: ACT square + DVE grouped reduce + fused sqrt(ss/64+eps) on ACT.
  - rc = rsqrt(vk)*rsqrt(vv) folded into V ONLY (vn = v*rc): the rotary muls
    k1 = k*cos, k2 = k*sin depend just on the PSUM copy, not the stats chain.
  - rotate-half swaps eliminated by reindexing (sigma involution): permuted
    sin tables + second PSUM accumulation region G; finalize computes
    dots[c] = D1[c] + G[sigma(c)] with 32-partition-block adds.
  - 1/N folded into the host q-path cos/sin tables.
  - dots emission lags LAG tiles: the deferred matmuls fill both the early
    DMA-bound gaps and the end-of-pass-1 elementwise-chain bubble; the last
    tile runs per-ci stats chains and defers its t12 muls.
  - tile-15 dots eb-major with per-eb finalize; pass 2 eb-major with
    A-matmuls leading S-matmuls by 2; output groups start small so the
    output-DMA pipe begins draining early, and end small to cut the tail.
"""

import numpy as np
import ml_dtypes

B, N, DIM, H, DH = 4, 8192, 512, 8, 64
HG = 2              # head groups (cores per batch) / head-pairs per core
HPG = H // HG       # heads per group = 4
E = HPG * DH        # 256 output cols per core
EPS = 1e-5
NT = 16             # n-tiles of 512
CPT = 4             # 128-chunks per n-tile
NCHUNK = NT * CPT   # 64
CC = DIM // 128     # 4 contraction chunks

_cache = {}


def _build_program():
    import concourse.tile as tile
    from concourse import bacc, mybir
    from contextlib import ExitStack

    f32 = mybir.dt.float32
    bf16 = mybir.dt.bfloat16

    nc = bacc.Bacc(target_bir_lowering=False)
    uxT = nc.declare_dram_parameter("uxT", [NT, 128, (CC + 2) * 512], bf16, isOutput=False)
    wq = nc.declare_dram_parameter("wq", [128, CC * E], bf16, isOutput=False)
    wkv = nc.declare_dram_parameter("wkv", [128, CC * 2 * E], bf16, isOutput=False)
    cosP = nc.declare_dram_parameter("cosP", [128, NCHUNK * DH], bf16, isOutput=False)
    sinNs = nc.declare_dram_parameter("sinNs", [128, NCHUNK * DH], bf16, isOutput=False)
    outT = nc.declare_dram_parameter("outT", [HG, 128, N], bf16, isOutput=True)

    with ExitStack() as ctx:
        tc = ctx.enter_context(tile.TileContext(nc))
        consts = ctx.enter_context(tc.tile_pool(name="consts", bufs=1))
        store = ctx.enter_context(tc.tile_pool(name="store", bufs=1))
        dpsp = ctx.enter_context(tc.tile_pool(name="dps", bufs=1, space="PSUM"))

        wq_sb = consts.tile([128, CC, E], bf16)
        wkv_sb = consts.tile([128, CC, 2 * E], bf16)
        nc.sync.dma_start(wkv_sb[:].rearrange("p c e -> p (c e)"), wkv[:])
        cosP_sb = consts.tile([128, NCHUNK, DH], bf16)
        sinNs_sb = consts.tile([128, NCHUNK, DH], bf16)

        eps_sb = consts.tile([128, 1], f32)
        nc.vector.memset(eps_sb[:], EPS)

        # t12T[a]: a=0 -> (q * cos/N)^T, a=1 -> (q * sinTsw/N)^T
        t12T_sb = store.tile([128, 2, HG, N], bf16)
        dotsA_sb = store.tile([128, HG, 128], bf16)  # block-diag D1+Sg(G)
        dotsS_sb = store.tile([128, HG, 128], bf16)  # sigma-row-permuted dots
        gsb = store.tile([128, HG, 128], f32)        # G staged out of PSUM

        # dots PSUM: [which(D1/G), eb, 128] -- one 2KB bank, 4 accumulation
        # regions. Outer level so finalize(eb1) overlaps pass-2(eb0).
        dots = dpsp.tile([128, 2, HG, 128], f32)

        def finalize(eb):
            # dotsA[c] = D1[c] + G[sigma(c)] (diag blocks only), dotsS[c] =
            # dotsA[sigma(c)]; sigma = 32-partition-block swap after _PI.
            nc.scalar.copy(gsb[:, eb, :], dots[:, 1, eb, :])
            for hh in range(2):          # head-in-pair: rows/cols 64-block
                r0, c0 = hh * 64, hh * 64
                for b in range(2):       # 32-partition sigma blocks
                    rb = r0 + b * 32
                    rs = r0 + (b ^ 1) * 32   # sigma partner block
                    cols = slice(c0, c0 + 64)
                    nc.vector.tensor_add(
                        dotsA_sb[rb:rb + 32, eb, cols],
                        dots[rb:rb + 32, 0, eb, cols],
                        gsb[rs:rs + 32, eb, cols])
            for b in range(4):
                rb, rs = b * 32, (b ^ 1) * 32
                nc.scalar.copy(dotsS_sb[rb:rb + 32, eb, :],
                               dotsA_sb[rs:rs + 32, eb, :])

        LAG = 6   # dots emission lag: deferred PE work fills pipeline bubbles

        with ExitStack() as p1:
            uxp = p1.enter_context(tc.tile_pool(name="uxp", bufs=4))
            work = p1.enter_context(tc.tile_pool(name="work", bufs=3))
            dwork = p1.enter_context(tc.tile_pool(name="dwork", bufs=LAG + 1))
            stats = p1.enter_context(tc.tile_pool(name="stats", bufs=3))
            qps = p1.enter_context(tc.tile_pool(name="qps", bufs=3, space="PSUM"))
            kps = p1.enter_context(tc.tile_pool(name="kps", bufs=4, space="PSUM"))

            nc.vector.memset(dotsA_sb[:], 0.0)

            # ---- DMA issue order (program order == dependency order): ----
            ux_tiles = {}

            def issue_ux(t):
                tl = uxp.tile([128, CC + 2, 512], bf16, tag="ux")
                nc.sync.dma_start(tl[:].rearrange("p c n -> p (c n)"), uxT[t, :, :])
                ux_tiles[t] = tl

            issue_ux(0)
            nc.sync.dma_start(wq_sb[:].rearrange("p c e -> p (c e)"), wq[:])
            issue_ux(1)
            nc.sync.dma_start(cosP_sb[:].rearrange("p t d -> p (t d)"), cosP[:])
            nc.sync.dma_start(sinNs_sb[:].rearrange("p t d -> p (t d)"), sinNs[:])
            issue_ux(2)

            pend = {}

            # NOTE: start=True zeroes the WHOLE PSUM bank -- only the very
            # first matmul into the bank carries it.
            def emit_dots(t):
                k1t, k2t, vxt = pend.pop(t)
                order = ([(ci, eb) for ci in range(CPT) for eb in range(HG)]
                         if t < NT - 1 else
                         [(ci, eb) for eb in range(HG) for ci in range(CPT)])
                for ci, eb in order:
                    gc = t * CPT + ci
                    last = gc == NCHUNK - 1
                    vx = vxt[:, ci, 2 * eb:2 * eb + 2, :]
                    nc.tensor.matmul(
                        dots[:, 0, eb, :], k1t[:, ci, 2 * eb:2 * eb + 2, :],
                        vx, start=(gc == 0 and eb == 0), stop=last)
                    nc.tensor.matmul(
                        dots[:, 1, eb, :], k2t[:, ci, 2 * eb:2 * eb + 2, :],
                        vx, start=False, stop=last)
                    if t == NT - 1 and last:
                        finalize(eb)

            for nt in range(NT):
                ns = nt * 512
                if nt + 3 < NT:
                    issue_ux(nt + 3)
                ux_t = ux_tiles.pop(nt)

                # ---- k/v projections; kv5 = [a(k/v), ci, g, d] ----
                kv5 = work.tile([128, 2, CPT, HPG, DH], bf16, tag="kv5")
                for ci in range(CPT):
                    kvp = kps.tile([128, 2 * E], f32, tag="kvp")
                    for cc in range(CC):
                        nc.tensor.matmul(
                            kvp[:], ux_t[:, cc, ci * 128:(ci + 1) * 128],
                            wkv_sb[:, cc, :],
                            start=(cc == 0), stop=(cc == CC - 1))
                    nc.scalar.copy(
                        kv5[:, :, ci, :, :],
                        kvp[:].rearrange("p (a g d) -> p a g d", a=2, g=HPG))

                # ---- transposed q projection + rotary premul; the last
                #      tile's t12 DVE muls are deferred until after the stats
                #      chain so vn(15) clears the DVE queue sooner ----
                t12_defer = []
                for eb in range(HG):
                    qp = qps.tile([128, 512], f32, tag="qp")
                    for cc in range(CC):
                        nc.tensor.matmul(
                            qp[:], wq_sb[:, cc, eb * 128:(eb + 1) * 128],
                            ux_t[:, cc, :],
                            start=(cc == 0), stop=(cc == CC - 1))
                    if nt < NT - 1:
                        nc.vector.tensor_mul(
                            t12T_sb[:, :, eb, ns:ns + 512],
                            qp[:].unsqueeze(1).broadcast_to([128, 2, 512]),
                            ux_t[:, CC:CC + 2, :])
                    else:
                        t12_defer.append((qp, eb))

                # ---- rotary muls on unnormalized k (no rc dependency) ----
                tsl = slice(nt * CPT, (nt + 1) * CPT)
                k1 = dwork.tile([128, CPT, HPG, DH], bf16, tag="k1")
                nc.vector.tensor_mul(
                    k1[:], kv5[:, 0],
                    cosP_sb[:, tsl, :].unsqueeze(2).broadcast_to(
                        [128, CPT, HPG, DH]))
                k2n = dwork.tile([128, CPT, HPG, DH], bf16, tag="k2n")
                nc.gpsimd.tensor_mul(
                    k2n[:], kv5[:, 0],
                    sinNs_sb[:, tsl, :].unsqueeze(2).broadcast_to(
                        [128, CPT, HPG, DH]))

                # ---- stats: ACT square, DVE grouped reduce, fused
                #      sqrt(ss/64+eps) on ACT; rc folded into v only.
                #      Last tile runs per-ci chains for short tail latency. ----
                sq5 = work.tile([128, 2, CPT, HPG, DH], bf16, tag="sq5")
                ss = stats.tile([128, 2, CPT, HPG], f32, tag="ss")
                ts = stats.tile([128, 2, CPT, HPG], f32, tag="ts")
                rc = stats.tile([128, CPT, HPG], f32, tag="rc")
                vn = dwork.tile([128, CPT, HPG, DH], bf16, tag="vn")
                if nt < NT - 1:
                    nc.scalar.square(sq5[:], kv5[:])
                    nc.vector.tensor_reduce(
                        out=ss[:], in_=sq5[:], axis=mybir.AxisListType.X,
                        op=mybir.AluOpType.add)
                    nc.scalar.activation(
                        ts[:].rearrange("p a c g -> p (a c g)"),
                        ss[:].rearrange("p a c g -> p (a c g)"),
                        mybir.ActivationFunctionType.Sqrt,
                        bias=eps_sb[:], scale=1.0 / DH)
                    sd = stats.tile([128, CPT * HPG], f32, tag="sd")
                    nc.vector.tensor_mul(
                        sd[:], ts[:, 0].rearrange("p c g -> p (c g)"),
                        ts[:, 1].rearrange("p c g -> p (c g)"))
                    nc.vector.reciprocal(rc[:].rearrange("p c g -> p (c g)"), sd[:])
                    nc.gpsimd.tensor_mul(
                        vn[:], kv5[:, 1],
                        rc[:].unsqueeze(-1).broadcast_to([128, CPT, HPG, DH]))
                else:
                    sd = stats.tile([128, CPT, HPG], f32, tag="sd")
                    for ci in range(CPT):
                        nc.scalar.square(sq5[:, :, ci], kv5[:, :, ci])
                        nc.vector.tensor_reduce(
                            out=ss[:, :, ci], in_=sq5[:, :, ci],
                            axis=mybir.AxisListType.X, op=mybir.AluOpType.add)
                        nc.scalar.activation(
                            ts[:, :, ci], ss[:, :, ci],
                            mybir.ActivationFunctionType.Sqrt,
                            bias=eps_sb[:], scale=1.0 / DH)
                        nc.vector.tensor_mul(sd[:, ci], ts[:, 0, ci],
                                             ts[:, 1, ci])
                        nc.vector.reciprocal(rc[:, ci], sd[:, ci])
                        nc.vector.tensor_mul(
                            vn[:, ci], kv5[:, 1, ci],
                            rc[:, ci].unsqueeze(-1).broadcast_to(
                                [128, HPG, DH]))
                for qp, eb in t12_defer:
                    nc.vector.tensor_mul(
                        t12T_sb[:, :, eb, ns:ns + 512],
                        qp[:].unsqueeze(1).broadcast_to([128, 2, 512]),
                        ux_t[:, CC:CC + 2, :])

                # ---- dots, LAG tiles behind (software pipeline) ----
                if nt >= LAG:
                    emit_dots(nt - LAG)
                pend[nt] = (k1, k2n, vn)
            for t in range(NT - LAG, NT):
                emit_dots(t)

        # ---- pass 2: uT[vfeat, tok] = dotsA^T@t1T + dotsS^T@t2T, eb-major
        #      so pass-2(eb0) overlaps finalize(eb1) ----
        with ExitStack() as p2:
            ups = p2.enter_context(tc.tile_pool(name="ups", bufs=6, space="PSUM"))
            uout = p2.enter_context(tc.tile_pool(name="uout", bufs=3))
            for eb in range(HG):
                # A-matmuls lead S-matmuls by 2; small leading groups start
                # the output-DMA pipe early, small trailing groups cut the
                # final drain.
                groups = [4, 4, 4, 4] if eb == 0 else [4, 4, 4, 2, 2]
                up_t = {}

                def a_mm(nt, eb=eb):
                    up = ups.tile([128, 512], f32, tag="up")
                    nc.tensor.matmul(up[:], dotsA_sb[:, eb, :],
                                     t12T_sb[:, 0, eb, nt * 512:(nt + 1) * 512],
                                     start=True, stop=False)
                    up_t[nt] = up

                a_mm(0)
                a_mm(1)
                nt = 0
                for gsz in groups:
                    ostage = uout.tile([128, gsz, 512], bf16, tag=f"os{gsz}")
                    for oi in range(gsz):
                        if nt + 2 < NT:
                            a_mm(nt + 2)
                        up = up_t.pop(nt)
                        nc.tensor.matmul(up[:], dotsS_sb[:, eb, :],
                                         t12T_sb[:, 1, eb, nt * 512:(nt + 1) * 512],
                                         start=False, stop=True)
                        dst = ostage[:, oi, :]
                        # DVE tensor_copy f32->bf16 rounding is broken; use
                        # tensor_scalar mult-1.0 on DVE, plain copy on ACT.
                        if nt % 2 == 0:
                            nc.vector.tensor_scalar(
                                out=dst, in0=up[:], scalar1=1.0, scalar2=None,
                                op0=mybir.AluOpType.mult)
                        else:
                            nc.scalar.copy(dst, up[:])
                        nt += 1
                    bs = (nt - gsz) * 512
                    nc.sync.dma_start(
                        outT[eb, :, bs:bs + gsz * 512],
                        ostage[:].rearrange("p o n -> p (o n)"))

    nc.finalize()
    return nc


def _center_heads(w):
    """InstanceNorm mean-subtraction folded into weights: per 64-row head
    block, subtract the block's column means."""
    wh = w.reshape(HPG, DH, DIM)
    return (wh - wh.mean(axis=1, keepdims=True)).reshape(E, DIM)


# rotary-axis permutation: pairs (c, sigma(c)) land 32 apart, so the sigma
# reindex in the kernel is a clean 32-partition-block swap
_PI = np.concatenate([np.arange(0, 16), np.arange(32, 48),
                      np.arange(16, 32), np.arange(48, 64)])


def _permq(w):
    """Permute per-head rotary output rows of a [E, DIM] projection weight."""
    return w.reshape(HPG, DH, DIM)[:, _PI, :].reshape(E, DIM)


def _host_prep(u_x, pos_x, Wq, Wk, Wv):
    bf = ml_dtypes.bfloat16
    invf = 1.0 / 10000.0 ** (np.arange(0, 32, dtype=np.float64)[::2] / 32)
    t64 = pos_x[0].astype(np.float64) * 64.0
    fx = t64[:, 0:1] * invf[None, :]
    fy = t64[:, 1:2] * invf[None, :]
    cx, sx = np.cos(fx), np.sin(fx)
    cy, sy = np.cos(fy), np.sin(fy)
    cosPf = np.concatenate([cx, cx, cy, cy], 1).astype(np.float32)[:, _PI]
    sinNf = -np.concatenate([sx, -sx, sy, -sy], 1).astype(np.float32)[:, _PI]

    sig64 = np.arange(64) ^ 32
    sinNsf = sinNf[:, sig64]          # d-permuted for the k-side G matmuls

    def chunked(t):  # [N, 64] -> [128, NCHUNK*64], partition = token % 128
        return np.ascontiguousarray(
            t.reshape(NCHUNK, 128, DH).transpose(1, 0, 2).reshape(128, -1)).astype(bf)

    cosP = chunked(cosPf)
    sinNs = chunked(sinNsf)

    # transposed q-path tables with 1/N folded in; sin rows sigma-permuted
    cosT = np.tile(cosPf.T, (2, 1)) / N            # [128, N]
    sinTsw = np.tile(sinNsf.T, (2, 1)) / N
    rot2c = np.ascontiguousarray(
        np.stack([cosT.reshape(128, NT, 512), sinTsw.reshape(128, NT, 512)],
                 axis=2).transpose(1, 0, 2, 3).reshape(NT, 128, 1024)).astype(bf)
    rot2f = rot2c.astype(np.float32)               # appended to uxT per tile

    def wlayout(wT):  # [512, E'] -> [128, CC*E'] partition-native
        Ep = wT.shape[1]
        return np.ascontiguousarray(
            wT.reshape(CC, 128, Ep).transpose(1, 0, 2).reshape(128, -1)).astype(bf)

    in_maps = []
    for b in range(B):
        uxtb = np.ascontiguousarray(np.concatenate([
            u_x[b].reshape(NT, 512, CC, 128).transpose(0, 3, 2, 1).reshape(
                NT, 128, CC * 512),
            rot2f], axis=2)).astype(bf)
        for hg in range(HG):
            sl = slice(hg * E, (hg + 1) * E)
            wk_c = _permq(_center_heads(Wk[sl]))
            wv_c = _center_heads(Wv[sl])
            in_maps.append({
                "uxT": uxtb,
                "wq": wlayout(_permq(Wq[sl]).T.astype(np.float32)),
                "wkv": wlayout(np.concatenate([wk_c.T, wv_c.T], 1)),
                "cosP": cosP, "sinNs": sinNs,
            })
    return in_maps


def kernel(u_x, pos_x, Wq, Wk, Wv, _trace=False, _trace_dir=None):
    from concourse.bass_utils import run_bass_kernel_spmd

    if "nc" not in _cache:
        _cache["nc"] = _build_program()
    nc = _cache["nc"]

    in_maps = _host_prep(
        np.asarray(u_x, np.float32), np.asarray(pos_x, np.float32),
        np.asarray(Wq, np.float32), np.asarray(Wk, np.float32),
        np.asarray(Wv, np.float32))

    kw = {}
    if _trace:
        kw = {"trace": True, "tmpdir": _trace_dir}
    res = run_bass_kernel_spmd(nc, in_maps, core_ids=list(range(8)), **kw)
    _cache["last_result"] = res

    out = np.empty((B, N, H * DH), np.float32)
    for i in range(8):
        b, hg = divmod(i, HG)
        oT = res.results[i]["outT"].astype(np.float32)   # [HG, 128, N]
        out[b, :, hg * E:(hg + 1) * E] = oT.reshape(HG * 128, N).T
    return out
